# revision 42
# baseline (speedup 1.0000x reference)
"""MixedQLinear Trainium2 kernel — token-sharded (data-parallel) version.

Computation (per reference):
  x2 = x[0]                                  (M=4096, IN_F=4096) fp16
  int_x = x2[:, int_indices]                 (M, 3840)
  fp_x  = x2[:, fp_indices]                  (M, 256)
  per-token asym quant of int_x to int4:  scale=(mx-mn)/15, zero=mn
  q = round((int_x-zero)/scale) - 8          in [-8,7]
  out = scale*w_scale*(q @ w_int.T) + (zero+8*scale)*reduced_w + fp_x@fp_w.T + bias

Strategy: shard TOKENS across the 8 cores (512 each); every core holds the
FULL weight set in SBUF (int4 weights as fp8 ~15.7MB, fp path fp8, scales).
Each core quantizes only its own 4 token tiles — 8x less DVE/ACT/transpose
work than out-feature sharding, and no collective at all.

Device-side per core, per 128-token tile (all quant work on DVE — no
cross-engine hops until the transpose):
  - min/max stats (DVE trees, scratch carved from the ya/qtb buffers),
  - DVE tensor_scalar with per-partition AP scalars emits
    ya = x*rs + (bq+1536) in f16: the f16 output cast IS the
    round-to-nearest-int (f16 ulp=1 on [1024,2048)), so no separate
    rounding pass; alpha and a constant ones row ride as extra columns,
  - DMA xbar transpose (128 x 3968) -> k-major,
  - the -1536 de-bias is folded into the f16->fp8 copy; the copy writes
    the DoubleRowSwInterleave stationary layout (pairs interleaved,
    tokens reversed per 256-col block),
  - per 512-wide out chunk (8 chunks): 15 fp8 DR-SwInterleave matmuls
    (int path, K=3840) into p0; 1 fp8 DR matmul (fp path, K=256) plus a
    K=2 fp16 matmul ([alpha; ones] x [reduced_w; bias]) into p1,
  - combine: out = (p0*wscale)*scale_t + p1  (DVE mul + affine_then_add).

DMA notes (measured): descriptors drain in global dispatch order across
all 16 queues (~100 desc/us), so xt0 is split 4-way at the stream head,
small consts next, and the 15.7MB wq load fans out as 16 sub-DMAs with
dispatch split across gpsimd (chunks 0-3) and sync (chunks 4-7).

Host side: column gather, int4 unpack, fp8 casts, k-major weight layout.
"""

import os
import sys

import numpy as np

for _p in ("/opt/trn_rl_repo",):
    if _p not in sys.path and os.path.isdir(_p):
        sys.path.insert(0, _p)

TOKENS = 4096
IN_F = 4096
OUT_F = 4096
FP_F = 256
INT_F = IN_F - FP_F          # 3840
NCORES = 8
TPC = TOKENS // NCORES       # 512 tokens per core
NT = TPC // 128              # 4 token tiles per core
KE = INT_F // 128            # 30 int k-tiles
NCH = 8                      # out chunks
CHW = OUT_F // NCH           # 512 outs per chunk
KPF = 512                    # padded fp-path contraction (256 fp + bias + pad)
C16 = 1536.0                 # f16 magic: cast of (v+1536) to f16 rounds v to int

_PROGRAM = None
LAST_RESULTS = None


def _ensure_ntff_hook():
    """Install the axon NTFF profiling hook if the image's antenv lacks it.

    Best-effort: profiling only; compile/run work without it.
    """
    import contextlib
    import ctypes
    import types

    try:
        try:
            import antenv.axon_hooks as hooks_mod
        except ImportError:
            import antenv

            hooks_mod = types.ModuleType("antenv.axon_hooks")
            _holder = {}
            hooks_mod.set_axon_ntff_profile_hook = (
                lambda hook: _holder.__setitem__("hook", hook))
            hooks_mod.get_axon_ntff_profile_hook = (
                lambda: _holder.get("hook"))
            sys.modules["antenv.axon_hooks"] = hooks_mod
            antenv.axon_hooks = hooks_mod

        if hooks_mod.get_axon_ntff_profile_hook() is not None:
            return
        so_path = "/opt/axon/libaxon_pjrt.so"
        if not os.path.exists(so_path):
            return
        lib = ctypes.CDLL(so_path)
        if not hasattr(lib, "axon_start_nrt_profile"):
            return
        lib.axon_start_nrt_profile.argtypes = [
            ctypes.POINTER(ctypes.c_int64), ctypes.c_size_t]
        lib.axon_start_nrt_profile.restype = ctypes.c_int64
        lib.axon_stop_nrt_profile.argtypes = [ctypes.c_char_p]
        lib.axon_stop_nrt_profile.restype = ctypes.c_int64

        @contextlib.contextmanager
        def _hook(output_dir, device_ids):
            import jax

            jax.devices()
            if device_ids:
                ids = (ctypes.c_int64 * len(device_ids))(*device_ids)
                rc = lib.axon_start_nrt_profile(ids, len(device_ids))
            else:
                rc = lib.axon_start_nrt_profile(None, 0)
            if rc != 0:
                raise RuntimeError(f"axon_start_nrt_profile rc={rc}")
            try:
                yield
            finally:
                n = lib.axon_stop_nrt_profile(str(output_dir).encode())
                print(f"ntff profile: {n} file(s) written to {output_dir}")

        hooks_mod.set_axon_ntff_profile_hook(_hook)
    except Exception:
        pass


def _build_program():
    import concourse.mybir as mybir
    import concourse.tile as tile
    from concourse import bacc

    f16 = mybir.dt.float16
    f32 = mybir.dt.float32
    f8 = mybir.dt.float8e4
    Alu = mybir.AluOpType
    DR = mybir.MatmulPerfMode.DoubleRow
    SWI = os.environ.get("BASS_SWI", "1") == "1"
    DRS = mybir.MatmulPerfMode.DoubleRowSwInterleave

    nc = bacc.Bacc(None, target_bir_lowering=False)

    x_own = nc.dram_tensor("x_own", [TPC, INT_F], f16, kind="ExternalInput")
    fpx8_d = nc.dram_tensor("fpx8", [128, 2, TPC], f8, kind="ExternalInput")
    wq_d = nc.dram_tensor("wq", [128, NCH, KE, CHW], f8, kind="ExternalInput")
    fpw8_d = nc.dram_tensor("fpw8", [128, 2, OUT_F], f8,
                            kind="ExternalInput")
    wsb_d = nc.dram_tensor("wsb", [128, OUT_F], f16, kind="ExternalInput")
    redwb_d = nc.dram_tensor("redwb", [2, OUT_F], f16, kind="ExternalInput")
    out_d = nc.dram_tensor("out", [TPC, OUT_F], f16, kind="ExternalOutput")

    with tile.TileContext(nc) as tc:
        with tc.tile_pool(name="consts", bufs=1) as consts, \
             tc.tile_pool(name="xin", bufs=2) as xin, \
             tc.tile_pool(name="qt8", bufs=2) as qt8, \
             tc.tile_pool(name="stp", bufs=4) as stp, \
             tc.tile_pool(name="outp", bufs=2) as outp, \
             tc.tile_pool(name="mp", bufs=1) as mp, \
             tc.tile_pool(name="ps0", bufs=3, space="PSUM") as ps0, \
             tc.tile_pool(name="ps1", bufs=3, space="PSUM") as ps1:

            # A single DMA queue moves only ~30 GB/s and a dma_start costs
            # ~0.6us of dispatch: split the first x tile across 4 queues,
            # keep the small consts next, then fan the 15.7MB wq load over
            # 16 sub-DMAs so all queues pull weights concurrently.
            xt0 = xin.tile([128, INT_F], f16, tag="xt")
            for j in range(4):
                nc.gpsimd.dma_start(
                    out=xt0[32 * j:32 * (j + 1), :],
                    in_=x_own[32 * j:32 * (j + 1), :])

            fpx8_s = consts.tile([128, 2, TPC], f8)
            nc.gpsimd.dma_start(out=fpx8_s[:, :, :], in_=fpx8_d[:, :, :])
            wsb_s = consts.tile([128, OUT_F], f16)
            nc.gpsimd.dma_start(out=wsb_s[:, :], in_=wsb_d[:, :])
            redwb_s = consts.tile([2, OUT_F], f16)
            nc.gpsimd.dma_start(out=redwb_s[:, :], in_=redwb_d[:, :])
            fpw8_s = consts.tile([128, 2, OUT_F], f8)
            nc.gpsimd.dma_start(out=fpw8_s[:, :, :], in_=fpw8_d[:, :, :])
            xt1 = xin.tile([128, INT_F], f16, tag="xt")
            for j in range(2):
                nc.gpsimd.dma_start(
                    out=xt1[64 * j:64 * (j + 1), :],
                    in_=x_own[128 + 64 * j:128 + 64 * (j + 1), :])
            wq_s = consts.tile([128, NCH, KE, CHW], f8)
            for c in range(NCH):
                # dispatch costs ~0.6us each and serializes per engine:
                # split the 16 sub-DMAs across gpsimd and sync so the late
                # chunks' transfers start ~5us earlier
                eng = nc.gpsimd if c < 4 else nc.sync
                eng.dma_start(
                    out=wq_s[:, c, :KE // 2, :], in_=wq_d[:, c, :KE // 2, :])
                eng.dma_start(
                    out=wq_s[:, c, KE // 2:, :], in_=wq_d[:, c, KE // 2:, :])

            # Manual double-buffers for ya (pre-transpose, biased quant) and
            # qtb (post-transpose) so the pad columns are zeroed once. ya
            # col INT_F+1 is a constant ones row (bias path rides the
            # transpose with the alpha column).
            ya_b = [consts.tile([128, (KE + 1) * 128], f16, name=f"ya{i}",
                                tag=f"ya{i}") for i in range(2)]
            qtb_b = [consts.tile([128, (KE + 1) * 128], f16, name=f"qt{i}",
                                 tag=f"qt{i}") for i in range(2)]
            for i in range(2):
                nc.vector.memset(ya_b[i][:, INT_F + 1:INT_F + 2], 1.0)
                nc.vector.memset(ya_b[i][:, INT_F + 2:], 0.0)

            ppack = consts.tile([128, 4 * NT], f32)
            negc = consts.tile([128, 1], f32)
            nc.vector.memset(negc[:, :], -C16)
            onec = consts.tile([128, 1], f32)
            nc.vector.memset(onec[:, :], 1.0)

            def producer(r, xt):
                ya = ya_b[r % 2]
                qtb = qtb_b[r % 2]
                # min tree (scratch carved from ya; overwritten by ACT after)
                mn = stp.tile([128, 1], f32, tag="mn")
                mx = stp.tile([128, 1], f32, tag="mx")
                nc.vector.tensor_tensor(
                    out=ya[:, :1920], in0=xt[:, :1920], in1=xt[:, 1920:],
                    op=Alu.min)
                nc.vector.tensor_tensor(
                    out=ya[:, 1920:2880], in0=ya[:, :960], in1=ya[:, 960:1920],
                    op=Alu.min)
                nc.vector.tensor_reduce(
                    out=mn[:, :], in_=ya[:, 1920:2880],
                    axis=mybir.AxisListType.X, op=Alu.min)
                # max tree (scratch carved from qtb; overwritten by transpose)
                nc.vector.tensor_tensor(
                    out=qtb[:, :1920], in0=xt[:, :1920], in1=xt[:, 1920:],
                    op=Alu.max)
                nc.vector.tensor_tensor(
                    out=qtb[:, 1920:2880], in0=qtb[:, :960],
                    in1=qtb[:, 960:1920], op=Alu.max)
                nc.vector.tensor_reduce(
                    out=mx[:, :], in_=qtb[:, 1920:2880],
                    axis=mybir.AxisListType.X, op=Alu.max)
                # params: [scale, rs, bq+1536, alpha] at ppack[:, 4r:4r+4]
                d = stp.tile([128, 1], f32, tag="d")
                nc.vector.tensor_sub(d[:, :], mx[:, :], mn[:, :])
                nc.vector.tensor_scalar(
                    out=ppack[:, 4 * r:4 * r + 1], in0=d[:, :],
                    scalar1=1.0 / 15.0, scalar2=1e-8, op0=Alu.mult,
                    op1=Alu.max)
                nc.vector.reciprocal(
                    ppack[:, 4 * r + 1:4 * r + 2], ppack[:, 4 * r:4 * r + 1])
                tt = stp.tile([128, 1], f32, tag="tt")
                nc.vector.tensor_mul(
                    tt[:, :], mn[:, :], ppack[:, 4 * r + 1:4 * r + 2])
                # bq' = -mn*rs - 8 + 1536
                nc.vector.tensor_scalar(
                    out=ppack[:, 4 * r + 2:4 * r + 3], in0=tt[:, :],
                    scalar1=-1.0, scalar2=C16 - 8.0, op0=Alu.mult, op1=Alu.add)
                # alpha = mn + 8*scale (zero-point term; rides un-scaled path)
                t8 = stp.tile([128, 1], f32, tag="t8")
                nc.vector.tensor_scalar(
                    out=t8[:, :], in0=ppack[:, 4 * r:4 * r + 1],
                    scalar1=8.0, scalar2=None, op0=Alu.mult)
                nc.vector.tensor_add(
                    ppack[:, 4 * r + 3:4 * r + 4], t8[:, :], mn[:, :])

                # quantize+round: f16 cast of x*rs + (bq+1536) is the RNE.
                # On DVE (per-partition AP scalars) right after the stats on
                # the same queue: no cross-engine semaphore hop, and f16 2x
                # mode beats the ACT identity (2.0us vs 3.6us).
                nc.vector.tensor_scalar(
                    out=ya[:, :INT_F], in0=xt[:, :],
                    scalar1=ppack[:, 4 * r + 1:4 * r + 2],
                    scalar2=ppack[:, 4 * r + 2:4 * r + 3],
                    op0=Alu.mult, op1=Alu.add)
                nc.vector.tensor_copy(
                    out=ya[:, INT_F:INT_F + 1],
                    in_=ppack[:, 4 * r + 3:4 * r + 4])
                # k-major transpose via DMA xbar
                nc.sync.dma_start_transpose(
                    out=qtb.rearrange("p (e t) -> p e t", e=KE + 1),
                    in_=ya[:, :])
                # de-bias fused into the fp8 copy (split DVE/ACT by parity)
                q8 = qt8.tile([128, KE, 128], f8)
                q8f = q8.rearrange("p e t -> p (e t)")
                if SWI:
                    # SwInterleave stationary layout: per 256-col block of
                    # k-pair e, col 2c+i = q'[token 127-c, ktile 2e+i]
                    # (pairs interleaved, tokens reversed) -> hardware
                    # loads the stationary contiguously.
                    q8v = q8f.rearrange("p (e c i) -> p e c i", e=KE // 2,
                                        i=2)
                    qin = qtb[:, :INT_F].rearrange(
                        "p (e i t) -> p e t i", e=KE // 2, i=2)
                    nc.vector.tensor_scalar(
                        out=q8v[:, :, :, :], in0=qin[:, :, ::-1, :],
                        scalar1=-C16, scalar2=None, op0=Alu.add)
                elif r % 2 == 0:
                    nc.vector.tensor_scalar(
                        out=q8f[:, :], in0=qtb[:, :INT_F], scalar1=-C16,
                        scalar2=None, op0=Alu.add)
                else:
                    nc.scalar.activation(
                        out=q8f[:, :], in_=qtb[:, :INT_F],
                        func=mybir.ActivationFunctionType.Identity,
                        bias=negc[:, :], scale=onec[:, :])
                return q8

            def consumer(r, q8):
                qtb = qtb_b[r % 2]
                t0 = r * 128
                q8f2 = q8.rearrange("p e t -> p (e t)")
                for c in range(NCH):
                    o0 = c * CHW
                    p0 = ps0.tile([128, CHW], f32)
                    for e in range(KE // 2):
                        if SWI:
                            stat = q8f2[:, 256 * e:256 * (e + 1)].rearrange(
                                "p (a b) -> p a b", a=2)
                            pm = DRS
                        else:
                            stat = q8[:, 2 * e:2 * e + 2, :]
                            pm = DR
                        nc.tensor.matmul(
                            p0[:, :], stat,
                            wq_s[:, c, 2 * e:2 * e + 2, :],
                            start=(e == 0), stop=(e == KE // 2 - 1),
                            perf_mode=pm)
                    p1 = ps1.tile([128, CHW], f32)
                    nc.tensor.matmul(
                        p1[:, :], fpx8_s[:, :, t0:t0 + 128],
                        fpw8_s[:, :, o0:o0 + CHW],
                        start=True, stop=False, perf_mode=DR)
                    # K=2: [alpha; ones] x [reduced_w; bias] (both rode the
                    # transpose as ya columns INT_F and INT_F+1)
                    nc.tensor.matmul(
                        p1[:, :], qtb[0:2, INT_F:INT_F + 128],
                        redwb_s[:, o0:o0 + CHW], start=False, stop=True)
                    m = mp.tile([128, CHW], f16)
                    nc.vector.tensor_mul(
                        m[:, :], p0[:, :], wsb_s[:, o0:o0 + CHW])
                    ot = outp.tile([128, CHW], f16)
                    nc.vector.affine_then_add(
                        out=ot[:, :], in0=m[:, :], in1=p1[:, :],
                        scale=ppack[:, 4 * r:4 * r + 1], bias=0.0)
                    nc.gpsimd.dma_start(
                        out=out_d[t0:t0 + 128, o0:o0 + CHW], in_=ot[:, :])

            # Software pipeline: producer runs LA=1 tile ahead of the
            # consumer. LA must stay < the ya/qtb/q8 buffer count (2).
            LA = 1
            made = {}
            for r in range(NT):
                if r == 0:
                    xt = xt0
                elif r == 1:
                    xt = xt1
                else:
                    xt = xin.tile([128, INT_F], f16, tag="xt")
                    for j in range(2):
                        nc.gpsimd.dma_start(
                            out=xt[64 * j:64 * (j + 1), :],
                            in_=x_own[r * 128 + 64 * j:
                                      r * 128 + 64 * (j + 1), :])
                made[r] = producer(r, xt)
                if r >= LA:
                    consumer(r - LA, made.pop(r - LA))
            for r in range(NT - LA, NT):
                consumer(r, made.pop(r))

    nc.finalize()
    return nc


def _get_program():
    global _PROGRAM
    if _PROGRAM is None:
        _PROGRAM = _build_program()
    return _PROGRAM


def _unpack_i4(w_packed):
    """(out, INT_F//2) uint8 -> (out, INT_F) int8; col 2k=low nibble, 2k+1=high."""
    lo = (w_packed & 0x0F).astype(np.int8)
    hi = ((w_packed >> 4) & 0x0F).astype(np.int8)
    lo = np.where(lo >= 8, lo - 16, lo)
    hi = np.where(hi >= 8, hi - 16, hi)
    w = np.empty((w_packed.shape[0], w_packed.shape[1] * 2), dtype=np.int8)
    w[:, 0::2] = lo
    w[:, 1::2] = hi
    return w


def _prep_inputs(x, int_weight, weights_scales, reduced_w, fp_weight, bias,
                 int_indices, fp_indices):
    import ml_dtypes
    f8np = ml_dtypes.float8_e4m3

    x2 = np.asarray(x, dtype=np.float16)[0]
    int_idx = np.asarray(int_indices).astype(np.int64)
    fp_idx = np.asarray(fp_indices).astype(np.int64)

    x_int = np.ascontiguousarray(x2[:, int_idx])            # (M, 3840) f16
    fp_xT = np.ascontiguousarray(x2[:, fp_idx].T)           # (256, M) f16

    w_int = _unpack_i4(np.asarray(int_weight))              # (OUT_F, 3840) int8
    wsc = np.asarray(weights_scales).astype(np.float16)     # (OUT_F, 1)
    redw = np.asarray(reduced_w).astype(np.float16)         # (1, OUT_F)
    fpW = np.asarray(fp_weight).astype(np.float16)          # (OUT_F, 256)
    b = np.asarray(bias).astype(np.float16)                 # (OUT_F,)

    # int weights: [p, chunk, ktile, out-in-chunk], value w[o, k], k=e*128+p
    wq = np.ascontiguousarray(
        w_int.T.reshape(KE, 128, NCH, CHW).transpose(1, 2, 0, 3)
    ).astype(f8np)

    # fp weights fp8, K=256 exactly: [p, i, o] holds fpW[o, k=i*128+p]
    fpw8 = np.ascontiguousarray(
        fpW.T.astype(np.float32).reshape(2, 128, OUT_F).transpose(1, 0, 2)
    ).astype(f8np)

    wsb = np.broadcast_to(wsc[:, 0][None, :], (128, OUT_F)).copy()
    # row 0: reduced_w (the alpha term), row 1: bias (multiplied by ones)
    redwb = np.ascontiguousarray(
        np.stack([redw[0].astype(np.float16), b], axis=0))   # (2, OUT_F)

    in_maps = []
    for c in range(NCORES):
        tok = slice(c * TPC, (c + 1) * TPC)
        x_ownc = np.ascontiguousarray(x_int[tok])
        fpx8 = np.ascontiguousarray(
            fp_xT[:, tok].astype(np.float32).reshape(2, 128, TPC)
            .transpose(1, 0, 2)).astype(f8np)
        in_maps.append({"x_own": x_ownc, "fpx8": fpx8, "wq": wq,
                        "fpw8": fpw8, "wsb": wsb, "redwb": redwb})
    return in_maps


def kernel(x, int_weight, weights_scales, reduced_w, fp_weight, bias,
           int_indices, fp_indices):
    global LAST_RESULTS
    from concourse.bass_utils import run_bass_kernel_spmd

    _ensure_ntff_hook()
    in_maps = _prep_inputs(x, int_weight, weights_scales, reduced_w,
                           fp_weight, bias, int_indices, fp_indices)
    nc = _get_program()
    res = run_bass_kernel_spmd(nc, in_maps, core_ids=list(range(NCORES)))
    LAST_RESULTS = res
    out = np.concatenate([res.results[c]["out"] for c in range(NCORES)],
                         axis=0)
    return out[None].astype(np.float16)
